# revision 16
# baseline (speedup 1.0000x reference)
"""Trainium2 Bass kernel for nn_BasicSupervisedModel_76733885710651.

Model: 2-layer GCN (PyG GCNConv x2, no nonlinearity between) -> sentence
scores -> top-k cutoff mask + BCE loss.

Key algebraic collapse: both convs are linear, so
    att = A @ (A @ (x @ (W1 @ W2))) + (b1 @ W2) * (A @ 1) + b2
with A = D^-1/2 (W_adj + I) D^-1/2.  The 1024-dim hidden layer never needs to
be materialized: everything reduces to scalar-per-node message passing.

Distribution: nodes (and their incident in-edges) are partitioned across the
8 NeuronCores by destination (graph/data parallel per the sharding hint);
weight matrices are replicated; per-apply halo exchange of the full scalar
node vector is an AllGather (the "halo" of a random graph is everything).

Per-edge gather/scatter on device is done with a routing pipeline:
  broadcast-AP expansion (DVE) -> local_scatter #1 (GPSIMD, per-partition) ->
  PE 128x128 block transposes (cross-partition hop) -> local_scatter #2 ->
  padded segment reduce (DVE).
Host side only computes index/layout tensors (sharding prep); all FP math
runs on device.
"""
import numpy as np

# ---- problem constants (hardcoded; harness provides exactly these shapes) ----
N = 20000
E = 320000
DX = 768
HID = 1024
S = 2000
P = 128
NCORES = 8
NQ = 160                  # padded q columns: node n <-> (n % P, n // P), 160*128=20480
QO = NQ // NCORES         # 20 owned q-cols per core
NODES_PER_CORE = P * QO   # 2560
MT = DX // P              # 6 m-tiles
HT = HID // P             # 8 k-tiles
XT = NODES_PER_CORE // P  # 20 n-tiles

_cache = {}


# --------------------------------------------------------------------------
# host-side index prep (sharding / layout only -- no float arithmetic)
# --------------------------------------------------------------------------

def _cumcount(key):
    """rank of each element within its equal-key group (vectorized)."""
    order = np.argsort(key, kind="stable")
    sk = key[order]
    starts = np.r_[0, np.flatnonzero(np.diff(sk)) + 1]
    lens = np.diff(np.r_[starts, len(sk)])
    runid = np.repeat(np.arange(len(starts)), lens)
    ranks = np.arange(len(sk)) - starts[runid]
    out = np.empty(len(sk), np.int64)
    out[order] = ranks
    return out


def host_prep(x, edge_index, edge_weight, y, W1, b1, W2, b2):
    r = edge_index[0].astype(np.int64)
    c = edge_index[1].astype(np.int64)
    w = edge_weight.astype(np.float32)

    core = c // NODES_PER_CORE
    # capacities (data-dependent, fixed at compile time)
    K = int(_cumcount(r * NCORES + core).max()) + 1          # copies per (node, core)
    KIN = int(_cumcount(c).max()) + 1                        # max in-degree
    C1 = NQ * K
    NB = -(-C1 // P)                                         # blocks (ceil)
    pi_key = core * P * P + (r % P) * P + (c % P)
    NB = max(NB, int(_cumcount(pi_key).max()) + 1)
    C1S = NB * P                                             # scatter-1 dst size
    assert C1 <= 2046 and C1S <= 2046, (K, NB)
    C2 = QO * KIN
    assert C2 <= 2046, (QO, KIN)
    KF = KIN
    CF = NQ * KF                                             # full-graph dest pad
    consts = dict(K=K, KIN=KIN, C1=C1, NB=NB, C1S=C1S, C2=C2, KF=KF, CF=CF)

    # full-graph dest-padded weights (for degree; identical on all cores)
    m_full = _cumcount(c)
    wdp = np.zeros((P, CF), np.float32)
    wdp[c % P, (c // P) * KF + m_full] = w

    # per-core routing tensors
    per_core = []
    for k in range(NCORES):
        sel = core == k
        rr, cc, ww, mm = r[sel], c[sel], w[sel], m_full[sel]
        p, q = rr % P, rr // P
        i, d = cc % P, (cc // P) - QO * k
        ke = _cumcount(rr)
        s = q * K + ke
        be = _cumcount(p * P + i)
        wslots = np.zeros((P, C1), np.float32)
        scat1 = np.full((P, C1S), -1, np.int16)
        scat2 = np.full((P, C1S), -1, np.int16)
        wslots[p, s] = ww
        scat1[p, s] = be * P + i
        scat2[i, be * P + p] = d * KIN + mm
        wdpo = np.zeros((P, C2), np.float32)
        wdpo[i, d * KIN + mm] = ww
        per_core.append((wslots, scat1, scat2, wdpo))

    # sentence mask / labels (y is an int index tensor -> host logic is legit)
    smask = np.zeros((P, 16), np.float32)
    for q in range(16):
        nn = q * P + np.arange(P)
        smask[:, q] = (nn < S).astype(np.float32)
    sneg = (smask - 1.0) * 1e30
    toneh = np.zeros((P, 16), np.float32)
    ypos = np.unique(y[y > 0])
    toneh[ypos % P, ypos // P] = 1.0
    kk = max(int((y >= 0).sum()), 1)
    # kth_largest: k_adj = floor((1-q)*(n_valid-1)) = kk-2 -> second output is
    # desc[k_adj+1] = desc[kk-1], the exact kk-th largest. kk==1 handled by a
    # max-reduce path at build time.
    quantile = 1.0 - (kk - 1.5) / (S - 1.0)

    # x slices (pad the tail core with zeros)
    xp = np.zeros((NCORES * NODES_PER_CORE, DX), np.float32)
    xp[:N] = x
    x_slices = [xp[k * NODES_PER_CORE:(k + 1) * NODES_PER_CORE] for k in range(NCORES)]

    in_maps = []
    for k in range(NCORES):
        wslots, scat1, scat2, wdpo = per_core[k]
        in_maps.append(dict(
            x=x_slices[k], W1=W1.astype(np.float32),
            W2=W2.astype(np.float32).reshape(HID, 1), b1=b1.astype(np.float32).reshape(HID),
            b2=b2.astype(np.float32).reshape(1, 1),
            wslots=wslots, scat1=scat1, scat2=scat2,
            wdp=wdp, wdpo=wdpo, smask=smask, sneg=sneg, toneh=toneh,
        ))
    return consts, in_maps, kk, quantile


# --------------------------------------------------------------------------
# device kernel
# --------------------------------------------------------------------------

def build_kernel(consts, quantile, kk, repeat=1):
    import concourse.bacc as bacc
    import concourse.mybir as mybir
    import concourse.tile as tile
    from concourse.masks import make_identity

    f16, f32, i16 = mybir.dt.float16, mybir.dt.float32, mybir.dt.int16
    A = mybir.AluOpType
    K, KIN, C1, NB, C1S, C2, KF, CF = (consts[k] for k in
                                       ("K", "KIN", "C1", "NB", "C1S", "C2", "KF", "CF"))

    nc = bacc.Bacc("TRN2", target_bir_lowering=False, debug=False,
                   num_devices=NCORES)
    x_in = nc.dram_tensor("x", [NODES_PER_CORE, DX], f32, kind="ExternalInput")
    W1_in = nc.dram_tensor("W1", [DX, HID], f32, kind="ExternalInput")
    W2_in = nc.dram_tensor("W2", [HID, 1], f32, kind="ExternalInput")
    b1_in = nc.dram_tensor("b1", [HID], f32, kind="ExternalInput")
    b2_in = nc.dram_tensor("b2", [1, 1], f32, kind="ExternalInput")
    ws_in = nc.dram_tensor("wslots", [P, C1], f32, kind="ExternalInput")
    s1_in = nc.dram_tensor("scat1", [P, C1S], i16, kind="ExternalInput")
    s2_in = nc.dram_tensor("scat2", [P, C1S], i16, kind="ExternalInput")
    wdp_in = nc.dram_tensor("wdp", [P, CF], f32, kind="ExternalInput")
    wdpo_in = nc.dram_tensor("wdpo", [P, C2], f32, kind="ExternalInput")
    smask_in = nc.dram_tensor("smask", [P, 16], f32, kind="ExternalInput")
    sneg_in = nc.dram_tensor("sneg", [P, 16], f32, kind="ExternalInput")
    toneh_in = nc.dram_tensor("toneh", [P, 16], f32, kind="ExternalInput")
    loss_out = nc.dram_tensor("loss", [1, 1], f32, kind="ExternalOutput")
    mask_out = nc.dram_tensor("mask", [P, NQ], f32, kind="ExternalOutput")

    with tile.TileContext(nc) as tc:
        with (
            tc.tile_pool(name="sbuf", bufs=1) as sb,
            tc.tile_pool(name="stream", bufs=3) as st,
            tc.tile_pool(name="xtp", bufs=1) as xtp,
            tc.tile_pool(name="psum", bufs=2, space="PSUM") as ps,
            tc.tile_pool(name="dram", bufs=1, space="DRAM") as dram,
        ):
            identf = sb.tile([P, P], f16, tag="identf")
            ident32 = sb.tile([P, P], f32, tag="ident32")
            make_identity(nc, identf[:])
            make_identity(nc, ident32[:])
            ones_col = sb.tile([P, 1], f32, tag="ones_col")
            ones_row = sb.tile([1, P], f32, tag="ones_row")
            one11 = sb.tile([1, 1], f32, tag="one11")
            nc.vector.memset(ones_col[:], 1.0)
            nc.vector.memset(ones_row[:], 1.0)
            nc.vector.memset(one11[:], 1.0)

            # ---------------- persistent loads ----------------
            wslots = sb.tile([P, C1], f32, tag="wslots")
            sc1 = sb.tile([P, C1S], i16, tag="sc1")
            sc2 = sb.tile([P, C1S], i16, tag="sc2")
            nc.sync.dma_start(out=wslots[:], in_=ws_in[:])
            nc.sync.dma_start(out=sc1[:], in_=s1_in[:])
            nc.sync.dma_start(out=sc2[:], in_=s2_in[:])
            smask = sb.tile([P, 16], f32, tag="smask")
            sneg = sb.tile([P, 16], f32, tag="sneg")
            toneh = sb.tile([P, 16], f32, tag="toneh")
            nc.sync.dma_start(out=smask[:], in_=smask_in[:])
            nc.sync.dma_start(out=sneg[:], in_=sneg_in[:])
            nc.sync.dma_start(out=toneh[:], in_=toneh_in[:])

            # W / bias loads
            w1t = [[sb.tile([P, P], f32, tag=f"w1t{mt}_{kt}", name=f"w1t{mt}_{kt}")
                    for kt in range(HT)] for mt in range(MT)]
            w2 = sb.tile([P, HT], f32, tag="w2")
            b1 = sb.tile([P, HT], f32, tag="b1")
            b2 = sb.tile([1, 1], f32, tag="b2")
            nc.sync.dma_start(out=w2[:], in_=W2_in[:].rearrange("(t p) o -> p (t o)", p=P))
            nc.sync.dma_start(out=b1[:], in_=b1_in[:].rearrange("(t p) -> p t", p=P))
            nc.sync.dma_start(out=b2[:], in_=b2_in[:])

            # xT tiles (persistent, fp32): 6 x [128, 2560]
            xt = [xtp.tile([P, NODES_PER_CORE], f32, tag=f"xt{mt}", name=f"xt{mt}")
                  for mt in range(MT)]

            def body(rep):
                sfx = f"_r{rep}"

                # -------- stage A: degree / dinv (independent of x) --------
                dinv = sb.tile([P, NQ], f32, tag="dinv")
                NCH = 4
                cw = CF // NCH
                assert CF % NCH == 0 and cw % KF == 0
                degpre = sb.tile([P, NQ], f32, tag="degpre")
                for j in range(NCH):
                    wch = st.tile([P, cw], f32, tag="wdpch")
                    nc.sync.dma_start(out=wch[:], in_=wdp_in[:, j * cw:(j + 1) * cw])
                    nc.vector.tensor_reduce(
                        out=degpre[:, j * (cw // KF):(j + 1) * (cw // KF)]
                            .rearrange("p (d u) -> p d u", u=1),
                        in_=wch[:].rearrange("p (d k) -> p d k", k=KF),
                        axis=mybir.AxisListType.X, op=A.add)

                def rsqrt1p(dst, src):
                    # dst = 1/sqrt(src + 1): DVE +1, DVE reciprocal, ACT Sqrt
                    nc.vector.tensor_scalar(out=dst[:], in0=src[:], scalar1=1.0,
                                            scalar2=None, op0=A.add)
                    nc.vector.reciprocal(out=dst[:], in_=dst[:])
                    nc.scalar.activation(out=dst[:], in_=dst[:],
                                         func=mybir.ActivationFunctionType.Sqrt,
                                         bias=0.0, scale=1.0)

                rsqrt1p(dinv, degpre)
                dinvo = sb.tile([P, QO], f32, tag="dinvo")
                wdpo = sb.tile([P, C2], f32, tag="wdpo")
                nc.sync.dma_start(out=wdpo[:], in_=wdpo_in[:])
                degown = sb.tile([P, QO], f32, tag="degown")
                nc.vector.tensor_reduce(
                    out=degown[:].rearrange("p (d u) -> p d u", u=1),
                    in_=wdpo[:].rearrange("p (d k) -> p d k", k=KIN),
                    axis=mybir.AxisListType.X, op=A.add)
                rsqrt1p(dinvo, degown)

                # -------- stage B: w = W1 @ W2  (PE transposes + matvecs) --------
                for mt in range(MT):
                    w1tile = st.tile([P, HID], f32, tag="w1tile")
                    nc.sync.dma_start(out=w1tile[:], in_=W1_in[mt * P:(mt + 1) * P, :])
                    for kt in range(HT):
                        pt = ps.tile([P, P], f32, space="PSUM", tag="pt")
                        nc.tensor.transpose(out=pt[:], in_=w1tile[:, kt * P:(kt + 1) * P],
                                            identity=ident32[:])
                        nc.scalar.copy(out=w1t[mt][kt][:], in_=pt[:])
                wsb = sb.tile([P, MT], f32, tag="wsb")
                for mt in range(MT):
                    pw = ps.tile([P, 1], f32, space="PSUM", tag="pv")
                    for kt in range(HT):
                        nc.tensor.matmul(out=pw[:], lhsT=w1t[mt][kt][:],
                                         rhs=w2[:, kt:kt + 1],
                                         start=(kt == 0), stop=(kt == HT - 1))
                    nc.scalar.copy(out=wsb[:, mt:mt + 1], in_=pw[:])
                # s1 = b1 . W2 ; broadcast to [P,1]
                bw = sb.tile([P, HT], f32, tag="bw")
                nc.vector.tensor_tensor(out=bw[:], in0=b1[:], in1=w2[:], op=A.mult)
                bw1 = sb.tile([P, 1], f32, tag="bw1")
                nc.vector.tensor_reduce(out=bw1[:], in_=bw[:],
                                        axis=mybir.AxisListType.X, op=A.add)
                ps1 = ps.tile([1, 1], f32, space="PSUM", tag="p1")
                nc.tensor.matmul(out=ps1[:], lhsT=bw1[:], rhs=ones_col[:],
                                 start=True, stop=True)
                s1sb = sb.tile([1, 1], f32, tag="s1sb")
                nc.scalar.copy(out=s1sb[:], in_=ps1[:])
                s1b = sb.tile([P, 1], f32, tag="s1b")
                pb = ps.tile([P, 1], f32, space="PSUM", tag="pv")
                nc.tensor.matmul(out=pb[:], lhsT=ones_row[:], rhs=s1sb[:],
                                 start=True, stop=True)
                nc.scalar.copy(out=s1b[:], in_=pb[:])
                s2b = sb.tile([P, 1], f32, tag="s2b")
                pb2 = ps.tile([P, 1], f32, space="PSUM", tag="pv")
                nc.tensor.matmul(out=pb2[:], lhsT=ones_row[:], rhs=b2[:],
                                 start=True, stop=True)
                nc.scalar.copy(out=s2b[:], in_=pb2[:])

                # -------- stage C: z = x @ w  (transpose x, then matvec) --------
                for nt in range(XT):
                    xtile = st.tile([P, DX], f32, tag="xtile")
                    nc.sync.dma_start(out=xtile[:], in_=x_in[nt * P:(nt + 1) * P, :])
                    for mt in range(MT):
                        pt = ps.tile([P, P], f32, space="PSUM", tag="pt")
                        nc.tensor.transpose(out=pt[:], in_=xtile[:, mt * P:(mt + 1) * P],
                                            identity=ident32[:])
                        nc.scalar.copy(out=xt[mt][:, nt * P:(nt + 1) * P], in_=pt[:])
                zown = sb.tile([P, QO], f32, tag="zown")
                for nt in range(XT):
                    pz = ps.tile([P, 1], f32, space="PSUM", tag="pv")
                    for mt in range(MT):
                        nc.tensor.matmul(out=pz[:],
                                         lhsT=xt[mt][:, nt * P:(nt + 1) * P],
                                         rhs=wsb[:, mt:mt + 1],
                                         start=(mt == 0), stop=(mt == MT - 1))
                    nc.scalar.copy(out=zown[:, nt:nt + 1], in_=pz[:])

                # -------- all-gather helper --------
                def allgather(own, name):
                    gin = dram.tile([P, QO], f32, tag=f"agin_{name}")
                    gout = dram.tile([NCORES, P, QO], f32, tag=f"agout_{name}")
                    nc.gpsimd.dma_start(out=gin[:], in_=own[:])
                    nc.gpsimd.collective_compute(
                        "AllGather", A.bypass,
                        replica_groups=[list(range(NCORES))],
                        ins=[gin.opt()], outs=[gout.opt()])
                    full = sb.tile([P, NQ], f32, tag=f"full_{name}")
                    nc.sync.dma_start(
                        out=full[:].rearrange("p (k q) -> p k q", k=NCORES),
                        in_=gout[:].rearrange("k p q -> p k q"))
                    return full

                # -------- routing apply --------
                def apply_A(y2d, own_in, name):
                    """returns dinvo*(scatter-sum + dinvo*own_in) as [P, QO] f32"""
                    msg = sb.tile([P, C1], f16, tag=f"msg")
                    y_b = y2d[:].rearrange("p (q u) -> p q u", u=1).to_broadcast([P, NQ, K])
                    nc.vector.tensor_tensor(
                        out=msg[:].rearrange("p (q u) -> p q u", u=K),
                        in0=y_b, in1=wslots[:].rearrange("p (q u) -> p q u", u=K),
                        op=A.mult)
                    routed = sb.tile([P, C1S], f16, tag="routed")
                    nc.gpsimd.local_scatter(out_ap=routed[:], data_ap=msg[:],
                                            idxs_ap=sc1[:, :C1], channels=P,
                                            num_elems=C1S, num_idxs=C1)
                    post = sb.tile([P, C1S], f16, tag="post")
                    for b in range(NB):
                        pt = ps.tile([P, P], f16, space="PSUM", tag="pt")
                        nc.tensor.transpose(out=pt[:], in_=routed[:, b * P:(b + 1) * P],
                                            identity=identf[:])
                        nc.scalar.copy(out=post[:, b * P:(b + 1) * P], in_=pt[:])
                    destpad = sb.tile([P, C2], f16, tag="destpad")
                    nc.gpsimd.local_scatter(out_ap=destpad[:], data_ap=post[:],
                                            idxs_ap=sc2[:], channels=P,
                                            num_elems=C2, num_idxs=C1S)
                    upre = sb.tile([P, QO], f32, tag=f"upre_{name}")
                    nc.vector.tensor_reduce(
                        out=upre[:].rearrange("p (d u) -> p d u", u=1),
                        in_=destpad[:].rearrange("p (d k) -> p d k", k=KIN),
                        axis=mybir.AxisListType.X, op=A.add)
                    tmp = sb.tile([P, QO], f32, tag=f"tmp_{name}")
                    nc.vector.tensor_tensor(out=tmp[:], in0=dinvo[:], in1=own_in[:], op=A.mult)
                    nc.vector.tensor_tensor(out=tmp[:], in0=tmp[:], in1=upre[:], op=A.add)
                    nc.vector.tensor_tensor(out=tmp[:], in0=tmp[:], in1=dinvo[:], op=A.mult)
                    return tmp

                # apply 1: u = A @ z ; v = u + s1
                z2d = allgather(zown, "z" + sfx)
                y1 = sb.tile([P, NQ], f32, tag="y1")
                nc.vector.tensor_tensor(out=y1[:], in0=z2d[:], in1=dinv[:], op=A.mult)
                u = apply_A(y1, zown, "u" + sfx)
                vown = sb.tile([P, QO], f32, tag="vown")
                nc.vector.tensor_tensor(out=vown[:], in0=u[:],
                                        in1=s1b[:].to_broadcast([P, QO]), op=A.add)

                # apply 2: att = A @ v + s2
                v2d = allgather(vown, "v" + sfx)
                y2 = sb.tile([P, NQ], f32, tag="y2")
                nc.vector.tensor_tensor(out=y2[:], in0=v2d[:], in1=dinv[:], op=A.mult)
                att_pre = apply_A(y2, vown, "att" + sfx)
                atto = sb.tile([P, QO], f32, tag="atto")
                nc.vector.tensor_tensor(out=atto[:], in0=att_pre[:],
                                        in1=s2b[:].to_broadcast([P, QO]), op=A.add)

                att2d = allgather(atto, "att2d" + sfx)

                # -------- tail: cutoff, mask, loss --------
                sentm = sb.tile([P, 16], f32, tag="sentm")
                nc.vector.tensor_tensor(out=sentm[:], in0=att2d[:, :16], in1=smask[:],
                                        op=A.mult)
                nc.vector.tensor_tensor(out=sentm[:], in0=sentm[:], in1=sneg[:],
                                        op=A.add)
                if kk >= 2:
                    co = sb.tile([1, 2], f32, tag="co")
                    nc.gpsimd.kth_largest(out_ap=co[:], in_ap=sentm[:], n_per_lane=16,
                                          k=max(8, kk + 2), quantile=quantile)
                    cut11 = co[:, 1:2]
                else:
                    # kk == 1: cutoff = max(sent)
                    mcol = sb.tile([P, 1], f32, tag="mcol")
                    nc.vector.tensor_reduce(out=mcol[:], in_=sentm[:],
                                            axis=mybir.AxisListType.X, op=A.max)
                    pmr = ps.tile([1, P], f32, space="PSUM", tag="p1")
                    nc.tensor.transpose(out=pmr[:], in_=mcol[:], identity=ident32[:])
                    mrow = sb.tile([1, P], f32, tag="mrow")
                    nc.scalar.copy(out=mrow[:], in_=pmr[:])
                    co1 = sb.tile([1, 1], f32, tag="co1")
                    nc.vector.tensor_reduce(out=co1[:], in_=mrow[:],
                                            axis=mybir.AxisListType.X, op=A.max)
                    cut11 = co1[:]
                cutb = sb.tile([P, 1], f32, tag="cutb")
                pc = ps.tile([P, 1], f32, space="PSUM", tag="pv")
                nc.tensor.matmul(out=pc[:], lhsT=ones_row[:], rhs=cut11,
                                 start=True, stop=True)
                nc.scalar.copy(out=cutb[:], in_=pc[:])
                maskt = sb.tile([P, NQ], f32, tag="maskt")
                nc.vector.tensor_tensor(out=maskt[:], in0=att2d[:],
                                        in1=cutb[:].to_broadcast([P, NQ]), op=A.is_ge)
                nc.sync.dma_start(out=mask_out[:], in_=maskt[:])

                # softplus(x) = max(x,0) + ln(1 + exp(-|x|)); exact 0 at the
                # -1e30 sentinel entries.
                sp = sb.tile([P, 16], f32, tag="sp")
                ax = sb.tile([P, 16], f32, tag="ax")
                nc.scalar.activation(out=ax[:], in_=sentm[:],
                                     func=mybir.ActivationFunctionType.Abs,
                                     bias=0.0, scale=1.0)
                nc.scalar.activation(out=ax[:], in_=ax[:],
                                     func=mybir.ActivationFunctionType.Exp,
                                     bias=0.0, scale=-1.0)
                nc.scalar.activation(out=ax[:], in_=ax[:],
                                     func=mybir.ActivationFunctionType.Ln,
                                     bias=1.0, scale=1.0)
                nc.vector.tensor_scalar(out=sp[:], in0=sentm[:], scalar1=0.0,
                                        scalar2=None, op0=A.max)
                nc.vector.tensor_tensor(out=sp[:], in0=sp[:], in1=ax[:], op=A.add)
                tts = sb.tile([P, 16], f32, tag="tts")
                nc.vector.tensor_tensor(out=tts[:], in0=toneh[:], in1=att2d[:, :16],
                                        op=A.mult)
                nc.vector.tensor_tensor(out=sp[:], in0=sp[:], in1=tts[:], op=A.subtract)
                lterm = sb.tile([P, 1], f32, tag="lterm")
                nc.vector.tensor_reduce(out=lterm[:], in_=sp[:],
                                        axis=mybir.AxisListType.X, op=A.add)
                pl = ps.tile([1, 1], f32, space="PSUM", tag="p1")
                nc.tensor.matmul(out=pl[:], lhsT=lterm[:], rhs=ones_col[:],
                                 start=True, stop=True)
                losss = sb.tile([1, 1], f32, tag="losss")
                nc.scalar.activation(out=losss[:], in_=pl[:],
                                     func=mybir.ActivationFunctionType.Copy,
                                     bias=0.0, scale=1.0 / S)
                nc.sync.dma_start(out=loss_out[:], in_=losss[:])

            for rep in range(repeat):
                body(rep)

    nc.compile()
    return nc


def _get_compiled(consts, quantile, kk, repeat=1):
    key = (tuple(sorted(consts.items())), quantile, kk, repeat)
    if key not in _cache:
        _cache[key] = build_kernel(consts, quantile, kk, repeat)
    return _cache[key]


def kernel(**inputs):
    from concourse import bass_utils
    args = {k: np.asarray(v) for k, v in inputs.items()}
    consts, in_maps, kk, quantile = host_prep(
        args["x"], args["edge_index"], args["edge_weight"], args["y"],
        args["W1"], args["b1"], args["W2"], args["b2"])
    nc = _get_compiled(consts, quantile, kk)
    res = bass_utils.run_bass_kernel_spmd(nc, in_maps, core_ids=list(range(NCORES)))
    out = res.results[0]
    loss = np.float32(out["loss"][0, 0])
    mask2d = out["mask"]
    n = np.arange(N)
    mask = mask2d[n % P, n // P] > 0.5
    return loss, mask


# revision 28
# speedup vs baseline: 14.8082x; 14.8082x over previous
"""Trainium2 Bass kernel for nn_BasicSupervisedModel_76733885710651.

Model: 2-layer GCN (PyG GCNConv x2, no nonlinearity between) -> sentence
scores -> top-k cutoff mask + BCE loss.

Key algebraic collapse: both convs are linear, so
    att = A @ (A @ (x @ (W1 @ W2))) + (b1 @ W2) * (A @ 1) + b2
with A = D^-1/2 (W_adj + I) D^-1/2.  The 1024-dim hidden never materializes:
everything reduces to scalar-per-node message passing with z = x @ (W1@W2).

Distribution (per the sharding hint): nodes and their incident in-edges are
partitioned across the 8 NeuronCores by destination; the small weight
matrices are replicated; the halo exchange per sparse-matrix apply is an
AllGather of the scalar node vector (for a random graph the halo is
everything), plus one tiny AllGather to broadcast the top-k cutoff.

Each A-apply on device: AllGather y (y = dinv*z at sources), replicate the
flat vector across partitions, one GPSIMD ap_gather pulls every edge's
source value into a dest-padded slot layout (gather order == slot order, so
no scatter is ever needed), multiply by the host-laid-out edge weights,
and reduce the fixed-width per-dest slot groups. Host computes index/layout
tensors only (sharding prep); all float arithmetic runs on device.
"""
import numpy as np

# ---- problem constants (hardcoded; harness provides exactly these shapes) ----
N = 20000
E = 320000
DX = 768
HID = 1024
S = 2000
P = 128
NCORES = 8
NQ = 160                  # padded q columns; node n <-> (n % P, n // P) per core
QO = NQ // NCORES         # 20 owned q-cols per core
NODES_PER_CORE = P * QO   # 2560
NTOT = NCORES * NODES_PER_CORE  # 20480 padded nodes
DPG = NODES_PER_CORE // 8       # 320 dests per gpsimd group

_cache = {}


def _cumcount(key):
    """rank of each element within its equal-key group (vectorized)."""
    order = np.argsort(key, kind="stable")
    sk = key[order]
    starts = np.r_[0, np.flatnonzero(np.diff(sk)) + 1]
    lens = np.diff(np.r_[starts, len(sk)])
    runid = np.repeat(np.arange(len(starts)), lens)
    ranks = np.arange(len(sk)) - starts[runid]
    out = np.empty(len(sk), np.int64)
    out[order] = ranks
    return out


def host_prep(x, edge_index, edge_weight, y, W1, b1, W2, b2):
    r = edge_index[0].astype(np.int64)
    c = edge_index[1].astype(np.int64)
    w = edge_weight.astype(np.float32)

    KIN = int(_cumcount(c).max()) + 1          # padded in-degree (data dependent)
    J = DPG * KIN                              # gather slots per gpsimd group
    consts = dict(KIN=KIN, J=J)
    assert J % 16 == 0 and NTOT <= 32768

    m = _cumcount(c)                           # slot within dest
    cl = c % NODES_PER_CORE                    # core-local dest id
    core = c // NODES_PER_CORE
    pc = cl % P                                # dest partition
    qc = cl // P                               # dest q-col
    g = pc // 16                               # gpsimd group
    rp = pc % 16                               # partition-within-group
    # slot j within group, ordered [rp][qc][k] so the diag-extract is regular
    j = rp * (QO * KIN) + qc * KIN + m

    per_core = []
    for k in range(NCORES):
        sel = core == k
        gidx = np.zeros((P, J // 16), np.int16)
        wg = np.zeros((P, 8, J), np.float16)   # logical [group, slot]
        wg2 = np.zeros((8, J), np.float32)
        # idx for output position j of group g lives at [16*g + j%16, j//16]
        jj, gg = j[sel], g[sel]
        gidx[16 * gg + jj % 16, jj // 16] = r[sel].astype(np.int16)
        wg2[gg, jj] = w[sel]
        # replicate each group's weights across its 16 partitions
        wgr = np.repeat(wg2, 16, axis=0).astype(np.float16)  # [128, J]
        # per-core dest-padded weights for degree (same slot layout)
        wdpo = np.zeros((P, QO * KIN), np.float32)
        wdpo[pc[sel], qc[sel] * KIN + m[sel]] = w[sel]
        per_core.append((gidx, wgr, wdpo))

    # sentence mask / labels (y is an int index tensor -> host logic is legit)
    # core-local layout: att_own[p, q] = att[2560*k + 128*q + p]
    smask0 = np.zeros((P, QO), np.float32)
    for q in range(16):
        nn = q * P + np.arange(P)
        smask0[:, q] = (nn < S).astype(np.float32)
    smask_other = np.zeros((P, QO), np.float32)
    smask_other[:, 0] = 1.0                    # keep kth_largest well-defined
    toneh0 = np.zeros((P, QO), np.float32)
    ypos = np.unique(y[y > 0])
    toneh0[ypos % P, ypos // P] = 1.0
    kk = max(int((y >= 0).sum()), 1)
    quantile = 1.0 - (kk - 1.5) / (S - 1.0)
    diag16 = np.zeros((P, 16), np.float32)
    diag16[np.arange(P), np.arange(P) % 16] = 1.0

    xp = np.zeros((NTOT, DX), np.float32)
    xp[:N] = x
    W1f = np.ascontiguousarray(W1, np.float32)
    W2f = np.ascontiguousarray(W2, np.float32).reshape(HID, 1)
    b1f = np.ascontiguousarray(b1, np.float32).reshape(HID)
    b2f = np.ascontiguousarray(b2, np.float32).reshape(1, 1)

    in_maps = []
    for k in range(NCORES):
        gidx, wgr, wdpo = per_core[k]
        smask = smask0 if k == 0 else smask_other
        sneg = (smask - 1.0) * 1e30
        in_maps.append(dict(
            x=xp[k * NODES_PER_CORE:(k + 1) * NODES_PER_CORE],
            W1=W1f, W2=W2f, b1=b1f, b2=b2f,
            gidx=gidx, wg=wgr, wdpo=wdpo, diag16=diag16,
            smask=smask, sneg=sneg,
            toneh=toneh0 if k == 0 else np.zeros((P, QO), np.float32),
        ))
    return consts, in_maps, kk, quantile


# --------------------------------------------------------------------------
# device kernel
# --------------------------------------------------------------------------

def build_kernel(consts, quantile, kk, repeat=1):
    import concourse.bacc as bacc
    import concourse.mybir as mybir
    import concourse.tile as tile

    f16, f32, i16 = mybir.dt.float16, mybir.dt.float32, mybir.dt.int16
    A = mybir.AluOpType
    AF = mybir.ActivationFunctionType
    KIN, J = consts["KIN"], consts["J"]

    nc = bacc.Bacc("TRN2", target_bir_lowering=False, debug=False,
                   num_devices=NCORES)
    x_in = nc.dram_tensor("x", [NODES_PER_CORE, DX], f32, kind="ExternalInput")
    W1_in = nc.dram_tensor("W1", [DX, HID], f32, kind="ExternalInput")
    W2_in = nc.dram_tensor("W2", [HID, 1], f32, kind="ExternalInput")
    b1_in = nc.dram_tensor("b1", [HID], f32, kind="ExternalInput")
    b2_in = nc.dram_tensor("b2", [1, 1], f32, kind="ExternalInput")
    gidx_in = nc.dram_tensor("gidx", [P, J // 16], i16, kind="ExternalInput")
    wg_in = nc.dram_tensor("wg", [P, J], f16, kind="ExternalInput")
    wdpo_in = nc.dram_tensor("wdpo", [P, QO * KIN], f32, kind="ExternalInput")
    smask_in = nc.dram_tensor("smask", [P, QO], f32, kind="ExternalInput")
    sneg_in = nc.dram_tensor("sneg", [P, QO], f32, kind="ExternalInput")
    toneh_in = nc.dram_tensor("toneh", [P, QO], f32, kind="ExternalInput")
    diag16_in = nc.dram_tensor("diag16", [P, 16], f32, kind="ExternalInput")
    loss_out = nc.dram_tensor("loss", [1, 1], f32, kind="ExternalOutput")
    mask_out = nc.dram_tensor("mask", [P, QO], f32, kind="ExternalOutput")

    with tile.TileContext(nc) as tc:
        with (
            tc.tile_pool(name="sbuf", bufs=1) as sb,
            tc.tile_pool(name="psum", bufs=2, space="PSUM") as ps,
            tc.tile_pool(name="dram", bufs=1, space="DRAM") as dram,
        ):
            ones_col = sb.tile([P, 1], f32, tag="ones_col")
            ones_row = sb.tile([1, P], f32, tag="ones_row")
            nc.vector.memset(ones_col[:], 1.0)
            nc.vector.memset(ones_row[:], 1.0)

            gidx = sb.tile([P, J // 16], i16, tag="gidx")
            wg = sb.tile([P, J], f16, tag="wg")
            wdpo = sb.tile([P, QO * KIN], f32, tag="wdpo")
            smask = sb.tile([P, QO], f32, tag="smask")
            sneg = sb.tile([P, QO], f32, tag="sneg")
            toneh = sb.tile([P, QO], f32, tag="toneh")
            nc.sync.dma_start(out=gidx[:], in_=gidx_in[:])
            nc.sync.dma_start(out=wg[:], in_=wg_in[:])
            nc.sync.dma_start(out=wdpo[:], in_=wdpo_in[:])
            nc.sync.dma_start(out=smask[:], in_=smask_in[:])
            nc.sync.dma_start(out=sneg[:], in_=sneg_in[:])
            nc.sync.dma_start(out=toneh[:], in_=toneh_in[:])
            diag16 = sb.tile([P, 16], f32, tag="diag16")
            nc.sync.dma_start(out=diag16[:], in_=diag16_in[:])
            w2k = sb.tile([P, HID // P], f32, tag="w2k")
            b1k = sb.tile([P, HID // P], f32, tag="b1k")
            b2s = sb.tile([1, 1], f32, tag="b2s")
            nc.sync.dma_start(out=w2k[:], in_=W2_in[:].rearrange("(t p) o -> p (t o)", p=P))
            nc.sync.dma_start(out=b1k[:], in_=b1_in[:].rearrange("(t p) -> p t", p=P))
            nc.sync.dma_start(out=b2s[:], in_=b2_in[:])

            def body(rep):
                sfx = f"_r{rep}"

                # ---- dinv for owned dests: 1/sqrt(1 + sum_in w) ----
                dinvo = sb.tile([P, QO], f32, tag="dinvo")
                nc.vector.tensor_reduce(
                    out=dinvo[:].rearrange("p (d u) -> p d u", u=1),
                    in_=wdpo[:].rearrange("p (d k) -> p d k", k=KIN),
                    axis=mybir.AxisListType.X, op=A.add)
                nc.vector.tensor_scalar(out=dinvo[:], in0=dinvo[:], scalar1=1.0,
                                        scalar2=None, op0=A.add)
                nc.vector.reciprocal(out=dinvo[:], in_=dinvo[:])
                nc.scalar.activation(out=dinvo[:], in_=dinvo[:], func=AF.Sqrt,
                                     bias=0.0, scale=1.0)

                # ---- w = W1 @ W2 (DVE mult + reduce), then broadcast ----
                w2b = sb.tile([P, HID], f32, tag="w2b")
                nc.sync.dma_start(
                    out=w2b[:],
                    in_=W2_in[:].rearrange("h o -> o h").to_broadcast([P, HID]))
                wcol = sb.tile([P, DX // P], f32, tag="wcol")
                TH = DX // P // 2
                for h in range(2):
                    w1sb = sb.tile([P, TH * HID], f32, tag="w1sb", name="w1sb")
                    nc.sync.dma_start(
                        out=w1sb[:].rearrange("p (t h) -> p t h", t=TH),
                        in_=W1_in[h * TH * P:(h + 1) * TH * P]
                            .rearrange("(t p) h -> p t h", p=P))
                    nc.vector.tensor_tensor(
                        out=w1sb[:].rearrange("p (t h) -> p t h", t=TH),
                        in0=w1sb[:].rearrange("p (t h) -> p t h", t=TH),
                        in1=w2b[:].rearrange("p (o h) -> p o h", o=1)
                            .to_broadcast([P, TH, HID]),
                        op=A.mult)
                    nc.vector.tensor_reduce(
                        out=wcol[:, h * TH:(h + 1) * TH]
                            .rearrange("p (t u) -> p t u", u=1),
                        in_=w1sb[:].rearrange("p (t h) -> p t h", t=TH),
                        axis=mybir.AxisListType.X, op=A.add)
                wflat = dram.tile([1, DX], f32, tag=f"wflat{sfx}")
                nc.sync.dma_start(out=wflat[:].rearrange("o (t p) -> p (o t)", p=P),
                                  in_=wcol[:])
                wb = sb.tile([P, DX], f32, tag="wb")
                nc.sync.dma_start(out=wb[:], in_=wflat[:].to_broadcast([P, DX]))

                # ---- s1 = b1 . W2 ; s2 = b2 ; broadcast to [P, 1] ----
                bw = sb.tile([P, HID // P], f32, tag="bw")
                nc.vector.tensor_tensor(out=bw[:], in0=b1k[:], in1=w2k[:], op=A.mult)
                bw1 = sb.tile([P, 1], f32, tag="bw1")
                nc.vector.tensor_reduce(out=bw1[:], in_=bw[:],
                                        axis=mybir.AxisListType.X, op=A.add)
                ps1 = ps.tile([1, 1], f32, space="PSUM", tag="p1")
                nc.tensor.matmul(out=ps1[:], lhsT=bw1[:], rhs=ones_col[:],
                                 start=True, stop=True)
                s1s = sb.tile([1, 1], f32, tag="s1s")
                nc.scalar.copy(out=s1s[:], in_=ps1[:])
                s1b = sb.tile([P, 1], f32, tag="s1b")
                pb = ps.tile([P, 1], f32, space="PSUM", tag="pv")
                nc.tensor.matmul(out=pb[:], lhsT=ones_row[:], rhs=s1s[:],
                                 start=True, stop=True)
                nc.scalar.copy(out=s1b[:], in_=pb[:])
                s2b = sb.tile([P, 1], f32, tag="s2b")
                pb2 = ps.tile([P, 1], f32, space="PSUM", tag="pv")
                nc.tensor.matmul(out=pb2[:], lhsT=ones_row[:], rhs=b2s[:],
                                 start=True, stop=True)
                nc.scalar.copy(out=s2b[:], in_=pb2[:])

                # ---- z = x @ w (DVE mult + grouped reduce) ----
                zown = sb.tile([P, QO], f32, tag="zown")
                QH = QO // 2
                for h in range(2):
                    xsb = sb.tile([P, QH * DX], f32, tag="xsb", name="xsb")
                    nc.sync.dma_start(
                        out=xsb[:].rearrange("p (t m) -> p t m", t=QH),
                        in_=x_in[h * QH * P:(h + 1) * QH * P]
                            .rearrange("(t p) m -> p t m", p=P))
                    nc.vector.tensor_tensor(
                        out=xsb[:].rearrange("p (t m) -> p t m", t=QH),
                        in0=xsb[:].rearrange("p (t m) -> p t m", t=QH),
                        in1=wb[:].rearrange("p (o m) -> p o m", o=1)
                            .to_broadcast([P, QH, DX]),
                        op=A.mult)
                    nc.vector.tensor_reduce(
                        out=zown[:, h * QH:(h + 1) * QH]
                            .rearrange("p (t u) -> p t u", u=1),
                        in_=xsb[:].rearrange("p (t m) -> p t m", t=QH),
                        axis=mybir.AxisListType.X, op=A.add)

                # ---- one A-apply via AllGather + ap_gather ----
                def apply_A(y_own, name):
                    """y_own [P, QO] f32 -> returns sum_{e into c} w_e * y[r_e]
                    for owned dests, as [P, QO] f32."""
                    gin = dram.tile([1, NODES_PER_CORE], f32, tag=f"agin_{name}")
                    gout = dram.tile([NCORES, 1, NODES_PER_CORE], f32,
                                     tag=f"agout_{name}")
                    nc.gpsimd.dma_start(
                        out=gin[:].rearrange("o (q p) -> p (o q)", p=P),
                        in_=y_own[:])
                    nc.gpsimd.collective_compute(
                        "AllGather", A.bypass,
                        replica_groups=[list(range(NCORES))],
                        ins=[gin.opt()], outs=[gout.opt()])
                    yrep = sb.tile([P, NTOT], f32, tag="yrep")
                    nc.sync.dma_start(
                        out=yrep[:],
                        in_=gout[:].rearrange("k o n -> o (k n)")
                            .to_broadcast([P, NTOT]))
                    gat = sb.tile([P, J], f32, tag="gat")
                    nc.gpsimd.ap_gather(
                        out_ap=gat[:], in_ap=yrep[:], idxs_ap=gidx[:],
                        channels=P, num_elems=NTOT, d=1, num_idxs=J)
                    nc.vector.tensor_tensor(out=gat[:], in0=gat[:], in1=wg[:],
                                            op=A.mult)
                    ug = sb.tile([P, 16 * QO], f32, tag="ug")
                    nc.vector.tensor_reduce(
                        out=ug[:].rearrange("p (r u) -> p r u", u=1),
                        in_=gat[:].rearrange("p (r k) -> p r k", k=KIN),
                        axis=mybir.AxisListType.X, op=A.add)
                    # diag-extract: partition p keeps sub-row r == p % 16
                    ugm = sb.tile([P, 16 * QO], f32, tag="ugm")
                    nc.vector.tensor_tensor(
                        out=ugm[:].rearrange("p (r q) -> p r q", r=16),
                        in0=ug[:].rearrange("p (r q) -> p r q", r=16),
                        in1=diag16[:].rearrange("p (r o) -> p r o", r=16)
                            .to_broadcast([P, 16, QO]),
                        op=A.mult)
                    usum = sb.tile([P, QO], f32, tag=f"usum_{name}")
                    nc.vector.tensor_reduce(
                        out=usum[:].rearrange("p (q u) -> p q u", u=1),
                        in_=ugm[:].rearrange("p (r q) -> p q r", r=16),
                        axis=mybir.AxisListType.X, op=A.add)
                    return usum

                # apply 1: u = dinvo*(gather-sum(y1) + y1_own); v = u + s1
                y1 = sb.tile([P, QO], f32, tag="y1")
                nc.vector.tensor_tensor(out=y1[:], in0=zown[:], in1=dinvo[:],
                                        op=A.mult)
                us1 = apply_A(y1, "u" + sfx)
                u = sb.tile([P, QO], f32, tag="u")
                nc.vector.tensor_tensor(out=u[:], in0=us1[:], in1=y1[:], op=A.add)
                nc.vector.tensor_tensor(out=u[:], in0=u[:], in1=dinvo[:], op=A.mult)
                vown = sb.tile([P, QO], f32, tag="vown")
                nc.vector.tensor_tensor(out=vown[:], in0=u[:],
                                        in1=s1b[:].to_broadcast([P, QO]), op=A.add)

                # apply 2: att = dinvo*(gather-sum(y2) + y2_own) + s2
                y2 = sb.tile([P, QO], f32, tag="y2")
                nc.vector.tensor_tensor(out=y2[:], in0=vown[:], in1=dinvo[:],
                                        op=A.mult)
                us2 = apply_A(y2, "att" + sfx)
                atto = sb.tile([P, QO], f32, tag="atto")
                nc.vector.tensor_tensor(out=atto[:], in0=us2[:], in1=y2[:], op=A.add)
                nc.vector.tensor_tensor(out=atto[:], in0=atto[:], in1=dinvo[:],
                                        op=A.mult)
                nc.vector.tensor_tensor(out=atto[:], in0=atto[:],
                                        in1=s2b[:].to_broadcast([P, QO]), op=A.add)

                # ---- tail: cutoff (core 0), broadcast, mask, loss ----
                sentm = sb.tile([P, QO], f32, tag="sentm")
                nc.vector.tensor_tensor(out=sentm[:], in0=atto[:], in1=smask[:],
                                        op=A.mult)
                nc.vector.tensor_tensor(out=sentm[:], in0=sentm[:], in1=sneg[:],
                                        op=A.add)
                if kk >= 2:
                    co = sb.tile([1, 2], f32, tag="co")
                    nc.gpsimd.kth_largest(out_ap=co[:], in_ap=sentm[:, :16],
                                          n_per_lane=16, k=max(8, kk + 2),
                                          quantile=quantile)
                    cut11 = co[:, 1:2]
                else:
                    mcol = sb.tile([P, 1], f32, tag="mcol")
                    nc.vector.tensor_reduce(out=mcol[:], in_=sentm[:, :16],
                                            axis=mybir.AxisListType.X, op=A.max)
                    import concourse.bass_isa as bass_isa
                    co1p = sb.tile([P, 1], f32, tag="co1p")
                    nc.gpsimd.partition_all_reduce(co1p[:], mcol[:], channels=P,
                                                   reduce_op=bass_isa.ReduceOp.max)
                    cut11 = co1p[:1, :]
                cin = dram.tile([1, 1], f32, tag=f"cin{sfx}")
                cout = dram.tile([NCORES, 1, 1], f32, tag=f"cout{sfx}")
                nc.gpsimd.dma_start(out=cin[:], in_=cut11)
                nc.gpsimd.collective_compute(
                    "AllGather", A.bypass,
                    replica_groups=[list(range(NCORES))],
                    ins=[cin.opt()], outs=[cout.opt()])
                cut0 = sb.tile([1, 1], f32, tag="cut0")
                nc.sync.dma_start(out=cut0[:], in_=cout[0, :, :1])
                cutb = sb.tile([P, 1], f32, tag="cutb")
                pc_ = ps.tile([P, 1], f32, space="PSUM", tag="pv")
                nc.tensor.matmul(out=pc_[:], lhsT=ones_row[:], rhs=cut0[:],
                                 start=True, stop=True)
                nc.scalar.copy(out=cutb[:], in_=pc_[:])
                maskt = sb.tile([P, QO], f32, tag="maskt")
                nc.vector.tensor_tensor(out=maskt[:], in0=atto[:],
                                        in1=cutb[:].to_broadcast([P, QO]),
                                        op=A.is_ge)
                nc.sync.dma_start(out=mask_out[:], in_=maskt[:])

                # loss = mean(softplus(sent) - t * sent) over the S sentences
                sp = sb.tile([P, QO], f32, tag="sp")
                ax = sb.tile([P, QO], f32, tag="ax")
                nc.scalar.activation(out=ax[:], in_=sentm[:], func=AF.Abs,
                                     bias=0.0, scale=1.0)
                nc.scalar.activation(out=ax[:], in_=ax[:], func=AF.Exp,
                                     bias=0.0, scale=-1.0)
                nc.scalar.activation(out=ax[:], in_=ax[:], func=AF.Ln,
                                     bias=1.0, scale=1.0)
                nc.vector.tensor_scalar(out=sp[:], in0=sentm[:], scalar1=0.0,
                                        scalar2=None, op0=A.max)
                nc.vector.tensor_tensor(out=sp[:], in0=sp[:], in1=ax[:], op=A.add)
                tts = sb.tile([P, QO], f32, tag="tts")
                nc.vector.tensor_tensor(out=tts[:], in0=toneh[:], in1=atto[:],
                                        op=A.mult)
                nc.vector.tensor_tensor(out=sp[:], in0=sp[:], in1=tts[:],
                                        op=A.subtract)
                lterm = sb.tile([P, 1], f32, tag="lterm")
                nc.vector.tensor_reduce(out=lterm[:], in_=sp[:],
                                        axis=mybir.AxisListType.X, op=A.add)
                pl = ps.tile([1, 1], f32, space="PSUM", tag="p1")
                nc.tensor.matmul(out=pl[:], lhsT=lterm[:], rhs=ones_col[:],
                                 start=True, stop=True)
                losss = sb.tile([1, 1], f32, tag="losss")
                nc.scalar.activation(out=losss[:], in_=pl[:], func=AF.Copy,
                                     bias=0.0, scale=1.0 / S)
                nc.sync.dma_start(out=loss_out[:], in_=losss[:])

            for rep in range(repeat):
                body(rep)

    nc.compile()
    return nc


def _get_compiled(consts, quantile, kk, repeat=1):
    key = (tuple(sorted(consts.items())), quantile, kk, repeat)
    if key not in _cache:
        _cache[key] = build_kernel(consts, quantile, kk, repeat)
    return _cache[key]


def kernel(**inputs):
    from concourse import bass_utils
    args = {k: np.asarray(v) for k, v in inputs.items()}
    consts, in_maps, kk, quantile = host_prep(
        args["x"], args["edge_index"], args["edge_weight"], args["y"],
        args["W1"], args["b1"], args["W2"], args["b2"])
    nc = _get_compiled(consts, quantile, kk)
    res = bass_utils.run_bass_kernel_spmd(nc, in_maps, core_ids=list(range(NCORES)))
    loss = np.float32(res.results[0]["loss"][0, 0])
    mask = np.zeros(N, bool)
    for k in range(NCORES):
        m2d = res.results[k]["mask"]  # [P, QO]
        base = k * NODES_PER_CORE
        cnt = min(NODES_PER_CORE, N - base)
        if cnt <= 0:
            break
        nn = np.arange(cnt)
        mask[base:base + cnt] = m2d[nn % P, nn // P] > 0.5
    return loss, mask
